# revision 1
# baseline (speedup 1.0000x reference)
"""Bilinear (softmax-free) multi-head attention on 8 TRN2 NeuronCores.

Math: for each batch b,
    out_b = x_b @ M_b,   M_b = sum_h Wq[h] @ (Wk[h].T @ (x_b.T x_b) @ Wv[h]) @ Wo[h]
since (Q K^T) V = Q (K^T V) and every projection is linear. This collapses the
O(L^2) attention into two L-sized GEMMs (G = x^T x and out = x @ M) plus a tiny
512x512 head-folding chain.

Distribution (SPMD, no collectives): core i handles batch b = i//4 and output
row chunk c = i%4. Each core streams the full x_b to build G redundantly
(cheaper than a cross-core all-reduce at this size), folds all 8 heads into M,
and computes/stores only its own 1024-row slice of out. The per-core x input is
row-rotated so the core's own chunk occupies rows 0..1024 (G is invariant to row
permutations), letting all 8 cores share one program.

Precision/perf notes:
- x ships as fp16 (values ~N(0,1); fp16 matmuls accumulate exactly into fp32
  PSUM) which halves the dominant DMA stream; everything downstream overflows
  fp16 so the chain runs in float32r (fp32 storage, ~11-bit multiply) at full
  PE rate. Measured end-to-end rel err ~2e-4.
- G exploits symmetry: only the upper-triangle 128-row blocks are computed;
  the lower blocks are PE-transposed mirrors (bitwise identical).
- The chain computes D_h^T = (G Wv)_h^T Wk_h via one N=256 window per head
  pair, assembles blockdiag(D_h^T), and folds all heads with dense 512-wide
  matmuls; W layouts are host-prepacked ((d,hk)/(hk,d)/(hk,o)) and the
  own-chunk x^T is host-transposed.
"""

import numpy as np

import concourse.tile as tile
from concourse import bacc, mybir
from concourse.bass_utils import run_bass_kernel_spmd
from concourse.masks import make_identity

F32 = mybir.dt.float32
F32R = mybir.dt.float32r
F16 = mybir.dt.float16

B, L, D = 2, 4096, 512
H, DK = 8, 64
CHUNK = 1024          # output rows per core
P = 128               # SBUF partitions
TL = L // P           # 32 x-tiles of 128 rows
NXD = 8               # x DMA chunks (4 x-tiles per chunk)
N_CORES = 8

_CACHE = {}


def _build():
    nc = bacc.Bacc("TRN2", target_bir_lowering=False, debug=False)

    x_d = nc.dram_tensor("x", [L, D], F16, kind="ExternalInput").ap()
    xt_d = nc.dram_tensor("xt", [D, CHUNK], F16, kind="ExternalInput").ap()
    wk_d = nc.dram_tensor("wk", [D, D], F16, kind="ExternalInput").ap()    # (d, h*k)
    wv_d = nc.dram_tensor("wv", [D, D], F16, kind="ExternalInput").ap()    # (d, h*k)
    wqt_d = nc.dram_tensor("wqt", [D, D], F32R, kind="ExternalInput").ap()  # (h*k, d)
    wo_d = nc.dram_tensor("wo", [D, D], F32R, kind="ExternalInput").ap()    # (h*k, o)
    out_d = nc.dram_tensor("out", [CHUNK, D], F16, kind="ExternalOutput").ap()

    with tile.TileContext(nc) as tc:
        import contextlib

        with contextlib.ExitStack() as ctx:
            consts = ctx.enter_context(tc.tile_pool(name="consts", bufs=1))
            wpool = ctx.enter_context(tc.tile_pool(name="wpool", bufs=1))
            xpool = ctx.enter_context(tc.tile_pool(name="xpool", bufs=1))
            spool = ctx.enter_context(tc.tile_pool(name="spool", bufs=1))
            opool = ctx.enter_context(tc.tile_pool(name="opool", bufs=6))
            pg = ctx.enter_context(tc.tile_pool(name="pg", bufs=4, space="PSUM"))
            pt = ctx.enter_context(tc.tile_pool(name="pt", bufs=4, space="PSUM"))

            # PE warmup: dependency-free dummy matmuls on an unwritten tile
            # start the Tensor engine's busy streak at t~0 so the pstate ramp
            # (0.65->1.2->2.4GHz over 3us of continuous busy) completes before
            # the first real G matmul; G then runs at full clock.
            wsrc = consts.tile([P, 256], F16, tag="warm")
            nc.vector.memset(wsrc[:], 0.0)
            wps = pg.tile([P, 64], F32, tag="acc", name="warm_ps")
            for _ in range(11):
                nc.tensor.matmul(
                    wps[:], lhsT=wsrc[:, 0:128], rhs=wsrc[:, 128:192],
                    start=True, stop=True,
                )

            ident_f32 = consts.tile([P, P], F32, tag="identf")
            make_identity(nc, ident_f32)
            ident = consts.tile([P, P], F16, tag="ident")
            nc.vector.tensor_copy(ident[:], ident_f32[:])

            # Zero-filled block-diag D^T holder, prepared off the critical path;
            # the FP phase later writes only the 8 diagonal 64x64 blocks.
            zero_f32 = consts.tile([P, 2048], F32, tag="zerof")
            nc.gpsimd.memset(zero_f32[:], 0.0)
            dtbd = spool.tile([P, 2048], F32R, tag="dt", name="dtbd")
            nc.vector.tensor_copy(dtbd[:], zero_f32[:])

            # --- x first: 8 SBUF tiles of [128, 2048]; tile j holds rows 512j..
            # x_sb[j][p, 512*tt + d] = x[128*(4j+tt) + p, d] ---
            xr = x_d.rearrange("(t p) d -> p t d", p=P)  # [128, 32, 512]
            x_sb = []
            for j in range(NXD):
                xt_ = xpool.tile([P, 2048], F16, tag=f"x{j}", name=f"x_sb{j}")
                if j == 0:  # split early chunks so the PE ramps sooner
                    for hh in range(4):
                        nc.sync.dma_start(
                            out=xt_.rearrange("p (t d) -> p t d", t=4)[:, hh:hh + 1, :],
                            in_=xr[:, hh:hh + 1, :],
                        )
                else:
                    for hh in range(2):
                        nc.sync.dma_start(
                            out=xt_.rearrange("p (t d) -> p t d", t=4)[:, 2 * hh:2 * hh + 2, :],
                            in_=xr[:, j * 4 + 2 * hh:j * 4 + 2 * hh + 2, :],
                        )
                x_sb.append(xt_)

            # --- weights (needed only from the B phase on):
            # W_sb[p, cb*512 + j] = W[128*cb + p, j] ---
            wk_sb = wpool.tile([P, 2048], F16, tag="wk", name="wk_sb")
            wv_sb = wpool.tile([P, 2048], F16, tag="wv", name="wv_sb")
            wqt_sb = wpool.tile([P, 2048], F32R, tag="wqt", name="wqt_sb")
            wo_sb = wpool.tile([P, 2048], F32R, tag="wo", name="wo_sb")
            for sb, dram in ((wv_sb, wv_d), (wk_sb, wk_d), (wo_sb, wo_d), (wqt_sb, wqt_d)):
                nc.sync.dma_start(
                    out=sb.rearrange("p (c j) -> p c j", c=4),
                    in_=dram.rearrange("(c p) j -> p c j", p=P),
                )

            def xtile(t):
                return x_sb[t // 4][:, (t % 4) * 512:(t % 4 + 1) * 512]


            # --- own-chunk x^T, host-prepared: xt_sb[p, 1024*kc + l] = x[l, 128kc+p]
            xt_sb = spool.tile([P, 4096], F16, tag="xt", name="xt_sb")
            nc.sync.dma_start(
                out=xt_sb.rearrange("p (kc l) -> p kc l", kc=4),
                in_=xt_d.rearrange("(kc p) l -> p kc l", p=P),
            )

            # --- G = x^T x (512x512, symmetric): row-block m computes only
            # columns >= 128m (fp16 matmuls have no min-N penalty); missing
            # lower blocks are mirrored via PE transpose afterwards. ---
            g_n0 = [0, 128, 256, 384]  # first computed column per m-block (fp16: any N)
            g_ps = []
            for m in range(4):
                g_ps.append(pg.tile([P, 512], F32, tag="acc", name=f"g_ps{m}"))
            # t-outer while DMA streams; the last 8 t-steps run m-outer so
            # g_ps[0] closes early and its copy/mirrors overlap G's tail
            for t in range(TL - 8):
                xt_ = xtile(t)
                for m in range(4):
                    n0 = g_n0[m]
                    nc.tensor.matmul(
                        g_ps[m][:, n0:512],
                        lhsT=xt_[:, m * P:(m + 1) * P],
                        rhs=xt_[:, n0:512],
                        start=(t == 0),
                        stop=False,
                    )
            for m in range(4):
                n0 = g_n0[m]
                for t in range(TL - 8, TL):
                    xt_ = xtile(t)
                    nc.tensor.matmul(
                        g_ps[m][:, n0:512],
                        lhsT=xt_[:, m * P:(m + 1) * P],
                        rhs=xt_[:, n0:512],
                        start=False,
                        stop=(t == TL - 1),
                    )
            g_sb = spool.tile([P, 2048], F16, tag="g", name="g_sb")

            def g_copy(m):
                n0 = g_n0[m]
                eng_copy = nc.vector.tensor_copy if m % 2 == 0 else nc.scalar.copy
                eng_copy(g_sb[:, m * 512 + n0:(m + 1) * 512], g_ps[m][:, n0:512])

            def g_mirror(mr, jc):
                # G[mr-block, jc-cols] = T(G[jc-block, mr-cols])
                mir_ps = pt.tile([P, 512], F16, tag="tp", name="mir_ps")
                nc.tensor.transpose(
                    mir_ps[:, 0:P],
                    g_sb[:, jc * 512 + mr * P:jc * 512 + (mr + 1) * P],
                    ident[:],
                )
                nc.vector.tensor_copy(
                    g_sb[:, mr * 512 + jc * P:mr * 512 + (jc + 1) * P],
                    mir_ps[:, 0:P],
                )

            g_copy(0)
            g_mirror(1, 0)
            g_mirror(2, 0)
            g_mirror(3, 0)
            g_copy(1)
            g_mirror(2, 1)
            g_mirror(3, 1)
            g_copy(2)
            g_mirror(3, 2)
            g_copy(3)

            # --- B = G @ Wv_all (512 x 512). lhsT uses G symmetry. ---
            b_ps = []
            for m in range(4):
                b_ps.append(pg.tile([P, 512], F32, tag="acc", name=f"b_ps{m}"))
            for m in range(4):
                for kc in range(4):
                    nc.tensor.matmul(
                        b_ps[m][:],
                        lhsT=g_sb[:, kc * 512 + m * P: kc * 512 + (m + 1) * P],
                        rhs=wv_sb[:, kc * 512:(kc + 1) * 512],
                        start=(kc == 0),
                        stop=(kc == 3),
                    )
            b_sb = spool.tile([P, 2048], F16, tag="b", name="b_sb")
            for m in range(4):
                # odd blocks on DVE: m3 closes last (B is m-outer) and its
                # copy gates the FP phase, so it goes on the faster engine
                eng_copy = nc.vector.tensor_copy if m % 2 == 1 else nc.scalar.copy
                eng_copy(b_sb[:, m * 512:(m + 1) * 512], b_ps[m][:])

            # --- FP = B^T @ Wk_all (512x512); diag 64-blocks are D_h^T,
            # copied into the pre-zeroed block-diag tile. ---
            ns_sb = spool.tile([P, 2048], F32R, tag="ns", name="ns_sb")

            def fp_phase(m):  # heads 2m (partitions 0:64), 2m+1 (64:128)
                # fp16 inputs run full rate at any N: only the 128 diag cols
                fp_ps = pt.tile([P, 128], F32, tag="tp", name="fp_ps")
                for kc in range(4):
                    nc.tensor.matmul(
                        fp_ps[:],
                        lhsT=b_sb[:, kc * 512 + m * P: kc * 512 + (m + 1) * P],
                        rhs=wk_sb[:, kc * 512 + 128 * m: kc * 512 + 128 * m + 128],
                        start=(kc == 0),
                        stop=(kc == 3),
                    )
                h0, h1 = 2 * m, 2 * m + 1
                nc.vector.tensor_copy(
                    dtbd[0:64, m * 512 + 64 * h0: m * 512 + 64 * h0 + 64],
                    fp_ps[0:64, 0:64],
                )
                nc.vector.tensor_copy(
                    dtbd[64:128, m * 512 + 64 * h1: m * 512 + 64 * h1 + 64],
                    fp_ps[64:128, 64:128],
                )

            def n_phase(m):
                # N_stack = blockdiag(D_h) @ Wo_stack: diagonal chunk only
                n_ps = pt.tile([P, 512], F32, tag="tp", name="n_ps")
                nc.tensor.matmul(
                    n_ps[:],
                    lhsT=dtbd[:, m * 512 + P * m: m * 512 + P * (m + 1)],
                    rhs=wo_sb[:, m * 512:(m + 1) * 512],
                    start=True,
                    stop=True,
                )
                if m == 0:
                    nc.vector.tensor_copy(ns_sb[:, m * 512:(m + 1) * 512], n_ps[:])
                else:
                    nc.scalar.copy(ns_sb[:, m * 512:(m + 1) * 512], n_ps[:])

            # --- M = WqT_stack^T-contract @ N_stack:  M[d, o] ---
            m_sb = spool.tile([P, 2048], F16, tag="m", name="m_sb")
            m_ps = []
            for m in range(4):
                m_ps.append(pg.tile([P, 512], F32, tag="acc", name=f"m_ps{m}"))

            # staggered so DVE copies land while the PE runs the next group
            fp_phase(0)
            fp_phase(1)
            n_phase(0)
            fp_phase(2)
            n_phase(1)
            fp_phase(3)
            n_phase(2)
            n_phase(3)
            # kc-outer for ns-copy slack; final kc wave m-outer so m_ps[0]
            # closes early and its copy overlaps the remaining M matmuls
            for kc in range(3):
                for m in range(4):
                    nc.tensor.matmul(
                        m_ps[m][:],
                        lhsT=wqt_sb[:, kc * 512 + m * P: kc * 512 + (m + 1) * P],
                        rhs=ns_sb[:, kc * 512:(kc + 1) * 512],
                        start=(kc == 0),
                        stop=False,
                    )
            for m in range(4):
                nc.tensor.matmul(
                    m_ps[m][:],
                    lhsT=wqt_sb[:, 3 * 512 + m * P: 3 * 512 + (m + 1) * P],
                    rhs=ns_sb[:, 3 * 512:4 * 512],
                    start=False,
                    stop=True,
                )
            for m in range(4):
                eng_copy = nc.vector.tensor_copy if m % 2 == 0 else nc.scalar.copy
                eng_copy(m_sb[:, m * 512:(m + 1) * 512], m_ps[m][:])

            # --- out chunk = x[0:1024] @ M. The last l-block accumulates in
            # two column halves so its first half stores while the second
            # half's matmuls still run (shorter final drain). ---
            for lb in range(CHUNK // P - 1):
                o_ps = pg.tile([P, 512], F32, tag="acc", name="o_ps")
                for kc in range(4):
                    nc.tensor.matmul(
                        o_ps[:],
                        lhsT=xt_sb[:, 1024 * kc + P * lb: 1024 * kc + P * (lb + 1)],
                        rhs=m_sb[:, kc * 512:(kc + 1) * 512],
                        start=(kc == 0),
                        stop=(kc == 3),
                    )
                o_sb = opool.tile([P, 512], F16, tag="o", name="o_sb")
                # alternate store engine so the PSUM->SBUF drains don't
                # serialize on DVE at the end of the out phase
                if lb % 2 == 0:
                    nc.vector.tensor_copy(o_sb[:], o_ps[:])
                else:
                    nc.scalar.copy(o_sb[:], o_ps[:])
                # spread DMA issues across sequencers: the ~650ns per-issue
                # cost serializes the tail if a single engine fires them all
                nc.sync.dma_start(out=out_d[lb * P:(lb + 1) * P, :], in_=o_sb[:])
            lb = CHUNK // P - 1
            o_sbl = opool.tile([P, 512], F16, tag="o", name="o_sbl")
            for hx in range(2):
                o_psh = pg.tile([P, 256], F32, tag="acc", name=f"o_psh{hx}")
                for kc in range(4):
                    nc.tensor.matmul(
                        o_psh[:],
                        lhsT=xt_sb[:, 1024 * kc + P * lb: 1024 * kc + P * (lb + 1)],
                        rhs=m_sb[:, kc * 512 + 256 * hx: kc * 512 + 256 * hx + 256],
                        start=(kc == 0),
                        stop=(kc == 3),
                    )
                (nc.vector.tensor_copy if hx == 0 else nc.scalar.copy)(
                    o_sbl[:, hx * 256:(hx + 1) * 256], o_psh[:]
                )
                nc.sync.dma_start(
                    out=out_d[lb * P:(lb + 1) * P, hx * 256:(hx + 1) * 256],
                    in_=o_sbl[:, hx * 256:(hx + 1) * 256],
                )

    nc.compile()
    return nc


def _get_nc():
    if "nc" not in _CACHE:
        _CACHE["nc"] = _build()
    return _CACHE["nc"]


def kernel(x, W_q, W_k, W_v, W_o):
    x = np.ascontiguousarray(np.asarray(x, np.float32))
    W_q = np.asarray(W_q, np.float32)
    W_k = np.asarray(W_k, np.float32)
    W_v = np.asarray(W_v, np.float32)
    W_o = np.asarray(W_o, np.float32)

    wk_all = np.ascontiguousarray(W_k.transpose(1, 0, 2).reshape(D, D).astype(np.float16))
    wv_all = np.ascontiguousarray(W_v.transpose(1, 0, 2).reshape(D, D).astype(np.float16))
    wqt = np.ascontiguousarray(W_q.transpose(0, 2, 1).reshape(D, D)) * 2.0**-24  # (hk, d), scaled for f16 M
    wo = np.ascontiguousarray(W_o.reshape(D, D))                          # (hk, o)

    nc = _get_nc()
    in_maps = []
    for i in range(N_CORES):
        b, c = divmod(i, 4)
        xb = np.roll(x[b], -c * CHUNK, axis=0).astype(np.float16)  # G is perm-invariant
        xt = np.ascontiguousarray(x[b, c * CHUNK:(c + 1) * CHUNK].T).astype(np.float16)  # (D, CHUNK)
        in_maps.append(
            {"x": np.ascontiguousarray(xb), "xt": xt, "wk": wk_all, "wv": wv_all,
             "wqt": wqt, "wo": wo}
        )

    res = run_bass_kernel_spmd(nc, in_maps, list(range(N_CORES)))

    out = np.empty((B, L, D), np.float32)
    for i in range(N_CORES):
        b, c = divmod(i, 4)
        out[b, c * CHUNK:(c + 1) * CHUNK] = res.results[i]["out"].astype(np.float32) * 2.0**24
    return out



# revision 3
# speedup vs baseline: 1.0223x; 1.0223x over previous
"""Bilinear (softmax-free) MHA on 8 TRN2 NeuronCores — fp8 DoubleRow, rev 8.

See kernel_v3 docstring for the math + fp8 strategy. Rev 4 restructures for
the TimelineSim device model:
  - fewer DMA instructions (HWDGE costs ~630ns each, serialized): x8 in 7
    chunk DMAs into ONE SBUF tile, weights as 4 single transfers in chain
    stage order (wv,wk,wo,wqt), xt digits as 2 transfers.
  - fully-fp8 chain: N stage runs fp8 (d8 blockdiag @ wo8 direct), killing
    the wo16 upcast. Measured numpy rel err 1.07e-2 (gate 2e-2).
  - PSUM->SBUF copies spread across DVE/ACT/Pool.
  - out phase: first 4 (digit,k2) groups lb-inner; last 2 lb-outer with
    per-pair [128,2,512] stores (4 stores, one o2 buffer each, no WAR).

Scale ledger: g8 = G*2^-6, b8 = B*2^-6, d8 = D*2^-12, ns8 = Ns*2^-17,
m8 digits = M*2^-24, out = out*2^-24 (host * 2^24).
"""

import numpy as np
import ml_dtypes

import concourse.tile as tile
from concourse import bacc, mybir
from concourse.bass_utils import run_bass_kernel_spmd
from concourse.masks import make_identity

F32 = mybir.dt.float32
F16 = mybir.dt.float16
F8 = mybir.dt.float8e4
DR = mybir.MatmulPerfMode.DoubleRow
COPY = mybir.ActivationFunctionType.Copy
NPF8 = ml_dtypes.float8_e4m3

B, L, D = 2, 4096, 512
H, DK = 8, 64
CHUNK = 1024
P = 128
TL = L // P            # 32 x-tiles of 128 rows
NT2 = TL // 2          # 16 DoubleRow passes
N_CORES = 8
N_WARM = 5
X_CHUNKS = [(0, 6), (6, 12), (12, 18), (18, 24), (24, 28), (28, 30), (30, 32)]

_CACHE = {}


def _build():
    nc = bacc.Bacc("TRN2", target_bir_lowering=False, debug=False)

    x8_d = nc.dram_tensor("x8", [L, D], F8, kind="ExternalInput").ap()
    xthi_d = nc.dram_tensor("xthi", [P, 4096], F8, kind="ExternalInput").ap()
    xtlo_d = nc.dram_tensor("xtlo", [P, 4096], F8, kind="ExternalInput").ap()
    wv_d = nc.dram_tensor("wv", [D, D], F8, kind="ExternalInput").ap()
    wk_d = nc.dram_tensor("wk", [D, D], F8, kind="ExternalInput").ap()
    wo_d = nc.dram_tensor("wo", [D, D], F8, kind="ExternalInput").ap()
    wqt_d = nc.dram_tensor("wqt", [D, D], F8, kind="ExternalInput").ap()
    out_d = nc.dram_tensor("out", [CHUNK, D], F16, kind="ExternalOutput").ap()

    with tile.TileContext(nc) as tc:
        import contextlib

        with contextlib.ExitStack() as ctx:
            consts = ctx.enter_context(tc.tile_pool(name="consts", bufs=1))
            wpool = ctx.enter_context(tc.tile_pool(name="wpool", bufs=1))
            xpool = ctx.enter_context(tc.tile_pool(name="xpool", bufs=1))
            spool = ctx.enter_context(tc.tile_pool(name="spool", bufs=1))
            opool = ctx.enter_context(tc.tile_pool(name="opool", bufs=5))
            pg = ctx.enter_context(tc.tile_pool(name="pg", bufs=4, space="PSUM"))
            pt = ctx.enter_context(tc.tile_pool(name="pt", bufs=4, space="PSUM"))

            # PE warmup: starts the pstate busy streak at t~0.
            warm = consts.tile([P, 512], F16, tag="warm")
            nc.vector.memset(warm[:], 0.0)
            # dummy ACT op at t~0 so the 1.3us LoadActFuncSet runs during the
            # DMA lead-in instead of blocking the first real scaled copy
            nc.scalar.activation(warm[:, 0:8], warm[:, 0:8], COPY, scale=1.0)
            wps = pg.tile([P, 512], F32, tag="acc", name="warm_ps")
            for _ in range(N_WARM):
                nc.tensor.matmul(wps[:], lhsT=warm[:, 0:P], rhs=warm[:],
                                 start=True, stop=True)

            ident_f32 = consts.tile([P, P], F32, tag="identf")
            make_identity(nc, ident_f32)
            ident8 = consts.tile([P, P], F8, tag="ident8")
            nc.vector.tensor_copy(ident8[:], ident_f32[:])

            # fp8 block-diag D^T holder, zero-filled off the critical path
            d8 = spool.tile([P, 4, 512], F8, tag="d8", name="d8_sb")
            nc.gpsimd.memset(d8[:], 0.0)

            # --- x8 stream into one [128, 32, 512] tile; G pass t2 reads
            # slice [:, 2t2:2t2+2, :], chunk boundaries are pass-aligned ---
            xr = x8_d.rearrange("(t p) d -> p t d", p=P)
            x8_sb = xpool.tile([P, TL, 512], F8, tag="x8", name="x8_sb")
            for a, b in X_CHUNKS:
                nc.sync.dma_start(out=x8_sb[:, a:b, :], in_=xr[:, a:b, :])

            # --- weights: 4 single fp8 transfers in chain-stage order ---
            wv_sb = wpool.tile([P, 4, 512], F8, tag="wv", name="wv_sb")
            wk_sb = wpool.tile([P, 4, 512], F8, tag="wk", name="wk_sb")
            wo_sb = wpool.tile([P, 4, 512], F8, tag="wo", name="wo_sb")
            wqt_sb = wpool.tile([P, 4, 512], F8, tag="wqt", name="wqt_sb")
            for sb, dram in ((wv_sb, wv_d), (wk_sb, wk_d), (wo_sb, wo_d), (wqt_sb, wqt_d)):
                nc.sync.dma_start(out=sb[:], in_=dram.rearrange("(c p) j -> p c j", p=P))

            # --- own-chunk x^T fp8 digits [p, kc2, j, l] ---
            xthi_sb = spool.tile([P, 2, 2, 1024], F8, tag="xthi", name="xthi_sb")
            xtlo_sb = spool.tile([P, 2, 2, 1024], F8, tag="xtlo", name="xtlo_sb")
            nc.sync.dma_start(
                out=xthi_sb[:], in_=xthi_d.rearrange("p (k j l) -> p k j l", k=2, j=2))
            nc.sync.dma_start(
                out=xtlo_sb[:], in_=xtlo_d.rearrange("p (k j l) -> p k j l", k=2, j=2))

            # --- G = x^T x, fp8 DoubleRow, upper-triangle blocks only ---
            g_n0 = [0, 128, 256, 384]
            g_ps = [pg.tile([P, 512], F32, tag="acc", name=f"g_ps{m}") for m in range(4)]
            for t2 in range(NT2 - 1):
                xp = x8_sb[:, 2 * t2:2 * t2 + 2, :]
                for m in range(4):
                    n0 = g_n0[m]
                    nc.tensor.matmul(
                        g_ps[m][:, n0:512],
                        lhsT=xp[:, :, m * P:(m + 1) * P], rhs=xp[:, :, n0:512],
                        start=(t2 == 0), stop=False, perf_mode=DR)
            xp = x8_sb[:, TL - 2:TL, :]
            for m in range(4):  # final pass m-outer: g_ps[m] closes in sequence
                n0 = g_n0[m]
                nc.tensor.matmul(
                    g_ps[m][:, n0:512],
                    lhsT=xp[:, :, m * P:(m + 1) * P], rhs=xp[:, :, n0:512],
                    start=False, stop=True, perf_mode=DR)

            g8 = spool.tile([P, 4, 512], F8, tag="g8", name="g8_sb")

            def g_copy(m, eng):
                n0 = g_n0[m]
                if eng == "dve":
                    nc.vector.tensor_scalar_mul(g8[:, m, n0:512], g_ps[m][:, n0:512], 2.0 ** -6)
                else:
                    nc.scalar.activation(g8[:, m, n0:512], g_ps[m][:, n0:512], COPY, scale=2.0 ** -6)

            def g_mirror(mr, jc, eng):
                mir_ps = pt.tile([P, P, 2], F8, tag="tp", name="mir_ps")
                nc.tensor.transpose(mir_ps[:, :, 0], g8[:, jc, mr * P:(mr + 1) * P], ident8[:])
                eng(g8[:, mr, jc * P:(jc + 1) * P], mir_ps[:, :, 0])

            # upper-left quadrant first so B's k2=0 pass unblocks early
            g_copy(0, "dve")
            g_copy(1, "act")
            g_mirror(1, 0, nc.vector.tensor_copy)
            g_copy(2, "act")
            g_copy(3, "dve")
            g_mirror(2, 0, nc.scalar.copy)
            g_mirror(3, 0, nc.vector.tensor_copy)
            g_mirror(2, 1, nc.scalar.copy)
            g_mirror(3, 1, nc.vector.tensor_copy)
            g_mirror(3, 2, nc.scalar.copy)

            # --- B = G @ Wv (fp8 DR), k2-outer so k2=0 runs on half of g8 ---
            b8 = spool.tile([P, 4, 512], F8, tag="b8", name="b8_sb")
            b_ps = [pg.tile([P, 512], F32, tag="acc", name=f"b_ps{m}") for m in range(4)]
            for k2 in range(2):
                for m in range(4):
                    nc.tensor.matmul(
                        b_ps[m][:],
                        lhsT=g8[:, 2 * k2:2 * k2 + 2, m * P:(m + 1) * P],
                        rhs=wv_sb[:, 2 * k2:2 * k2 + 2, :],
                        start=(k2 == 0), stop=(k2 == 1), perf_mode=DR)
            for m in range(4):
                if m % 2 == 0:
                    nc.vector.tensor_copy(b8[:, m, :], b_ps[m][:])
                else:
                    nc.scalar.copy(b8[:, m, :], b_ps[m][:])

            # --- FP = B^T @ Wk diag windows (fp8 DR) -> d8 = D*2^-12 ---
            def fp_phase(m):
                fp_ps = pt.tile([P, P], F32, tag="tp", name="fp_ps")
                for k2 in range(2):
                    nc.tensor.matmul(
                        fp_ps[:],
                        lhsT=b8[:, 2 * k2:2 * k2 + 2, m * P:(m + 1) * P],
                        rhs=wk_sb[:, 2 * k2:2 * k2 + 2, m * P:(m + 1) * P],
                        start=(k2 == 0), stop=(k2 == 1), perf_mode=DR)
                h0, h1 = 2 * m, 2 * m + 1
                nc.vector.tensor_scalar_mul(
                    d8[0:64, m, 64 * h0:64 * h0 + 64], fp_ps[0:64, 0:64], 2.0 ** -6)
                nc.scalar.activation(
                    d8[64:128, m, 64 * h1:64 * h1 + 64], fp_ps[64:128, 64:128],
                    COPY, scale=2.0 ** -6)

            # --- N = blockdiag(D) @ Wo, all fp8 (no upcast): Ns*2^-12 ---
            ns8 = spool.tile([P, 4, 512], F8, tag="ns8", name="ns8_sb")

            def n_phase(m):
                n_ps = pt.tile([P, 512], F32, tag="tp", name="n_ps")
                nc.tensor.matmul(
                    n_ps[:],
                    lhsT=d8[:, m, m * P:(m + 1) * P],
                    rhs=wo_sb[:, m, :],
                    start=True, stop=True)
                if m % 2 == 0:
                    nc.vector.tensor_scalar_mul(ns8[:, m, :], n_ps[:], 2.0 ** -5)
                else:
                    nc.scalar.activation(ns8[:, m, :], n_ps[:], COPY, scale=2.0 ** -5)

            for m in range(4):
                fp_phase(m)
            for m in range(4):
                n_phase(m)

            # --- M = Wqt^T @ Ns (fp8 DR), digits m8hi/m8lo = M*2^-24 ---
            m8hi = spool.tile([P, 4, 512], F8, tag="m8hi", name="m8hi_sb")
            m8lo = spool.tile([P, 4, 512], F8, tag="m8lo", name="m8lo_sb")
            m_ps = [pg.tile([P, 512], F32, tag="acc", name=f"m_ps{m}") for m in range(4)]
            for k2 in range(2):
                for m in range(4):
                    nc.tensor.matmul(
                        m_ps[m][:],
                        lhsT=wqt_sb[:, 2 * k2:2 * k2 + 2, m * P:(m + 1) * P],
                        rhs=ns8[:, 2 * k2:2 * k2 + 2, :],
                        start=(k2 == 0), stop=(k2 == 1), perf_mode=DR)
            for m in range(4):
                if m % 2 == 1:
                    nc.scalar.activation(m8hi[:, m, :], m_ps[m][:], COPY, scale=2.0 ** -7)
                else:
                    nc.vector.tensor_scalar_mul(m8hi[:, m, :], m_ps[m][:], 2.0 ** -7)
            for m in range(4):
                # DVE-only; issued before the o2 copies so they drain while
                # the out phase runs its m8hi-only groups
                nc.vector.scalar_tensor_tensor(
                    m8lo[:, m, :], m_ps[m][:], 2.0 ** -7, m8hi[:, m, :],
                    mybir.AluOpType.mult, mybir.AluOpType.subtract)

            # --- out = xh@mh + xl@mh + xh@ml (fp8 DR, 6 accum groups/lb) ---
            o_ps = []
            for lb in range(8):
                pool, tag = (pg, "acc") if lb < 4 else (pt, "tp")
                o_ps.append(pool.tile([P, 512], F32, tag=tag, name=f"o_ps{lb}"))

            # gi 0,1 = (hi,hi); 2,3 = (lo,hi); 4,5 = (hi,lo)
            groups = [
                (xthi_sb, m8hi, 0), (xthi_sb, m8hi, 1),
                (xtlo_sb, m8hi, 0), (xtlo_sb, m8hi, 1),
                (xthi_sb, m8lo, 0), (xthi_sb, m8lo, 1),
            ]

            def omm(lb, gi):
                xsrc, msrc, k2 = groups[gi]
                nc.tensor.matmul(
                    o_ps[lb][:],
                    lhsT=xsrc[:, k2, :, lb * P:(lb + 1) * P],
                    rhs=msrc[:, 2 * k2:2 * k2 + 2, :],
                    start=(gi == 0), stop=(gi == 5), perf_mode=DR)


            orr = out_d.rearrange("(q p) d -> p q d", p=P)  # [128, 8, 512]

            def finish_lb(lb, o_t, slot):
                for gi in range(6):
                    omm(lb, gi)
                if lb >= 6:  # split the last copies across both engines
                    nc.vector.tensor_copy(o_t[:, slot, 0:256], o_ps[lb][:, 0:256])
                    nc.scalar.copy(o_t[:, slot, 256:512], o_ps[lb][:, 256:512])
                elif lb % 2 == 0:
                    nc.vector.tensor_copy(o_t[:, slot, :], o_ps[lb][:])
                else:
                    nc.scalar.copy(o_t[:, slot, :], o_ps[lb][:])

            for pair in range(4):  # lb-major tail, stores trickle per pair
                o2 = opool.tile([P, 2, 512], F16, tag=f"o{pair}", name=f"o2_{pair}")
                finish_lb(2 * pair, o2, 0)
                finish_lb(2 * pair + 1, o2, 1)
                nc.sync.dma_start(out=orr[:, 2 * pair:2 * pair + 2, :], in_=o2[:])

    nc.compile()
    return nc


def _get_nc():
    if "nc" not in _CACHE:
        _CACHE["nc"] = _build()
    return _CACHE["nc"]


def _pack_xt_digits(xc):
    xt = np.ascontiguousarray(xc.T)                  # (D, CHUNK) f32
    hi = xt.astype(NPF8)
    lo = (xt - hi.astype(np.float32)).astype(NPF8)

    def pack(a):
        return np.ascontiguousarray(
            a.reshape(2, 2, P, CHUNK).transpose(2, 0, 1, 3).reshape(P, 4 * CHUNK))
    return pack(hi), pack(lo)


def kernel(x, W_q, W_k, W_v, W_o):
    x = np.ascontiguousarray(np.asarray(x, np.float32))
    W_q = np.asarray(W_q, np.float32)
    W_k = np.asarray(W_k, np.float32)
    W_v = np.asarray(W_v, np.float32)
    W_o = np.asarray(W_o, np.float32)

    wv8 = np.ascontiguousarray(W_v.transpose(1, 0, 2).reshape(D, D)).astype(NPF8)
    wk8 = np.ascontiguousarray(W_k.transpose(1, 0, 2).reshape(D, D)).astype(NPF8)
    wqt8 = np.ascontiguousarray(W_q.transpose(0, 2, 1).reshape(D, D)).astype(NPF8)
    wo8 = np.ascontiguousarray(W_o.reshape(D, D)).astype(NPF8)

    nc = _get_nc()
    in_maps = []
    for i in range(N_CORES):
        b, c = divmod(i, 4)
        xb = np.roll(x[b], -c * CHUNK, axis=0)
        x8 = np.ascontiguousarray(xb).astype(NPF8)
        xthi, xtlo = _pack_xt_digits(x[b, c * CHUNK:(c + 1) * CHUNK])
        in_maps.append(
            {"x8": x8, "xthi": xthi, "xtlo": xtlo,
             "wv": wv8, "wk": wk8, "wo": wo8, "wqt": wqt8}
        )

    res = run_bass_kernel_spmd(nc, in_maps, list(range(N_CORES)))

    out = np.empty((B, L, D), np.float32)
    for i in range(N_CORES):
        b, c = divmod(i, 4)
        out[b, c * CHUNK:(c + 1) * CHUNK] = res.results[i]["out"].astype(np.float32) * 2.0 ** 24
    return out


# revision 4
# speedup vs baseline: 1.0400x; 1.0174x over previous
"""Bilinear (softmax-free) multi-head attention on 8 TRN2 NeuronCores.

Math: for each batch b,
    out_b = x_b @ M_b,   M_b = sum_h Wq[h] @ (Wk[h].T @ (x_b.T x_b) @ Wv[h]) @ Wo[h]
since (Q K^T) V = Q (K^T V) and every projection is linear: the O(L^2)
attention collapses to one L-sized Gram matrix G = x^T x, a tiny 512x512
head-folding chain, and one L-sized output GEMM.

Distribution (SPMD, no collectives): core i handles batch b = i//4 and output
row chunk c = i%4. Each core streams the full x_b to build G redundantly,
folds all 8 heads into M, and computes/stores only its own 1024-row out
slice; the per-core x is row-rotated so all 8 cores share one program.

fp8 (e4m3) strategy — DoubleRow perf mode (2 stacked k-tiles per matmul,
0.5 PE cycles/output column, 4x fp16 MAC rate):
  - G: 16 DoubleRow passes of 256 rows, upper-triangle blocks only; lower
    blocks mirrored by PE transpose (fp8 transpose writes element-step-2).
  - chain B = G@Wv, FP = B^T@Wk (diag windows -> D_h^T), N = blockdiag(D)@Wo,
    M = Wqt^T@Ns: all fp8 (B/FP/M DoubleRow).
  - out phase: two-digit fp8: x_chunk^T and M each split into hi+lo fp8
    digits (lo = requantized residual); out = xh@mh + xl@mh + xh@ml.
  - end-to-end rel err ~9.1e-3 vs the 2e-2 gate.

Schedule (TimelineSim-driven): x8 chunks big-first so the DMA stream is
gapless and G (supply-bound) ends right behind the last chunk; weights
follow in chain-stage order; ACT's activation table is preloaded at t~0;
FP phases all precede N phases so d8 copies aren't queued behind ns8; the
out phase runs fully lb-major so PSUM->SBUF drains + pair stores trickle
out behind the matmuls.

Scale ledger (all power-of-2, folded host-side or into scaled copies):
g8 = G*2^-6, b8 = B*2^-6, d8 = D*2^-12, ns8 = Ns*2^-17,
m8 digits = M*2^-24, out = out*2^-24 (host multiplies back 2^24).
"""

import numpy as np
import ml_dtypes

import concourse.tile as tile
from concourse import bacc, mybir
from concourse.bass_utils import run_bass_kernel_spmd
from concourse.masks import make_identity

F32 = mybir.dt.float32
F16 = mybir.dt.float16
F8 = mybir.dt.float8e4
DR = mybir.MatmulPerfMode.DoubleRow
COPY = mybir.ActivationFunctionType.Copy
NPF8 = ml_dtypes.float8_e4m3

B, L, D = 2, 4096, 512
H, DK = 8, 64
CHUNK = 1024
P = 128
TL = L // P            # 32 x-tiles of 128 rows
NT2 = TL // 2          # 16 DoubleRow passes
N_CORES = 8
N_WARM = 5
X_CHUNKS = [(0, 6), (6, 12), (12, 18), (18, 24), (24, 28), (28, 30), (30, 32)]

_CACHE = {}


def _build():
    nc = bacc.Bacc("TRN2", target_bir_lowering=False, debug=False)

    x8_d = nc.dram_tensor("x8", [L, D], F8, kind="ExternalInput").ap()
    xthi_d = nc.dram_tensor("xthi", [P, 4096], F8, kind="ExternalInput").ap()
    xtlo_d = nc.dram_tensor("xtlo", [P, 4096], F8, kind="ExternalInput").ap()
    wv_d = nc.dram_tensor("wv", [D, D], F8, kind="ExternalInput").ap()
    wk_d = nc.dram_tensor("wk", [D, D], F8, kind="ExternalInput").ap()
    wo_d = nc.dram_tensor("wo", [D, D], F8, kind="ExternalInput").ap()
    wqt_d = nc.dram_tensor("wqt", [D, D], F8, kind="ExternalInput").ap()
    out_d = nc.dram_tensor("out", [CHUNK, D], F16, kind="ExternalOutput").ap()

    with tile.TileContext(nc) as tc:
        import contextlib

        with contextlib.ExitStack() as ctx:
            consts = ctx.enter_context(tc.tile_pool(name="consts", bufs=1))
            wpool = ctx.enter_context(tc.tile_pool(name="wpool", bufs=1))
            xpool = ctx.enter_context(tc.tile_pool(name="xpool", bufs=1))
            spool = ctx.enter_context(tc.tile_pool(name="spool", bufs=1))
            opool = ctx.enter_context(tc.tile_pool(name="opool", bufs=5))
            pg = ctx.enter_context(tc.tile_pool(name="pg", bufs=4, space="PSUM"))
            pt = ctx.enter_context(tc.tile_pool(name="pt", bufs=4, space="PSUM"))

            # PE warmup: starts the pstate busy streak at t~0.
            warm = consts.tile([P, 512], F16, tag="warm")
            nc.vector.memset(warm[:], 0.0)
            # dummy ACT op at t~0 so the 1.3us LoadActFuncSet runs during the
            # DMA lead-in instead of blocking the first real scaled copy
            nc.scalar.activation(warm[:, 0:8], warm[:, 0:8], COPY, scale=1.0)
            wps = pg.tile([P, 512], F32, tag="acc", name="warm_ps")
            for _ in range(N_WARM):
                nc.tensor.matmul(wps[:], lhsT=warm[:, 0:P], rhs=warm[:],
                                 start=True, stop=True)

            ident_f32 = consts.tile([P, P], F32, tag="identf")
            make_identity(nc, ident_f32)
            ident8 = consts.tile([P, P], F8, tag="ident8")
            nc.vector.tensor_copy(ident8[:], ident_f32[:])

            # fp8 block-diag D^T holder, zero-filled off the critical path
            d8 = spool.tile([P, 4, 512], F8, tag="d8", name="d8_sb")
            nc.gpsimd.memset(d8[:], 0.0)

            # --- x8 stream into one [128, 32, 512] tile; G pass t2 reads
            # slice [:, 2t2:2t2+2, :], chunk boundaries are pass-aligned ---
            xr = x8_d.rearrange("(t p) d -> p t d", p=P)
            x8_sb = xpool.tile([P, TL, 512], F8, tag="x8", name="x8_sb")
            for a, b in X_CHUNKS:
                nc.sync.dma_start(out=x8_sb[:, a:b, :], in_=xr[:, a:b, :])

            # --- weights: 4 single fp8 transfers in chain-stage order ---
            wv_sb = wpool.tile([P, 4, 512], F8, tag="wv", name="wv_sb")
            wk_sb = wpool.tile([P, 4, 512], F8, tag="wk", name="wk_sb")
            wo_sb = wpool.tile([P, 4, 512], F8, tag="wo", name="wo_sb")
            wqt_sb = wpool.tile([P, 4, 512], F8, tag="wqt", name="wqt_sb")
            for sb, dram in ((wv_sb, wv_d), (wk_sb, wk_d), (wo_sb, wo_d), (wqt_sb, wqt_d)):
                nc.sync.dma_start(out=sb[:], in_=dram.rearrange("(c p) j -> p c j", p=P))

            # --- own-chunk x^T fp8 digits [p, kc2, j, l] ---
            xthi_sb = spool.tile([P, 2, 2, 1024], F8, tag="xthi", name="xthi_sb")
            xtlo_sb = spool.tile([P, 2, 2, 1024], F8, tag="xtlo", name="xtlo_sb")
            nc.sync.dma_start(
                out=xthi_sb[:], in_=xthi_d.rearrange("p (k j l) -> p k j l", k=2, j=2))
            nc.sync.dma_start(
                out=xtlo_sb[:], in_=xtlo_d.rearrange("p (k j l) -> p k j l", k=2, j=2))

            # --- G = x^T x, fp8 DoubleRow, upper-triangle blocks only ---
            g_n0 = [0, 128, 256, 384]
            g_ps = [pg.tile([P, 512], F32, tag="acc", name=f"g_ps{m}") for m in range(4)]
            for t2 in range(NT2 - 1):
                xp = x8_sb[:, 2 * t2:2 * t2 + 2, :]
                for m in range(4):
                    n0 = g_n0[m]
                    nc.tensor.matmul(
                        g_ps[m][:, n0:512],
                        lhsT=xp[:, :, m * P:(m + 1) * P], rhs=xp[:, :, n0:512],
                        start=(t2 == 0), stop=False, perf_mode=DR)
            xp = x8_sb[:, TL - 2:TL, :]
            for m in range(4):  # final pass m-outer: g_ps[m] closes in sequence
                n0 = g_n0[m]
                nc.tensor.matmul(
                    g_ps[m][:, n0:512],
                    lhsT=xp[:, :, m * P:(m + 1) * P], rhs=xp[:, :, n0:512],
                    start=False, stop=True, perf_mode=DR)

            g8 = spool.tile([P, 4, 512], F8, tag="g8", name="g8_sb")

            def g_copy(m, eng):
                n0 = g_n0[m]
                if eng == "dve":
                    nc.vector.tensor_scalar_mul(g8[:, m, n0:512], g_ps[m][:, n0:512], 2.0 ** -6)
                else:
                    nc.scalar.activation(g8[:, m, n0:512], g_ps[m][:, n0:512], COPY, scale=2.0 ** -6)

            def g_mirror(mr, jc, eng):
                mir_ps = pt.tile([P, P, 2], F8, tag="tp", name="mir_ps")
                nc.tensor.transpose(mir_ps[:, :, 0], g8[:, jc, mr * P:(mr + 1) * P], ident8[:])
                eng(g8[:, mr, jc * P:(jc + 1) * P], mir_ps[:, :, 0])

            # upper-left quadrant first so B's k2=0 pass unblocks early
            g_copy(0, "dve")
            g_copy(1, "act")
            g_mirror(1, 0, nc.vector.tensor_copy)
            g_copy(2, "act")
            g_copy(3, "dve")
            g_mirror(2, 0, nc.scalar.copy)
            g_mirror(3, 0, nc.vector.tensor_copy)
            g_mirror(2, 1, nc.scalar.copy)
            g_mirror(3, 1, nc.vector.tensor_copy)
            g_mirror(3, 2, nc.scalar.copy)

            # --- B = G @ Wv (fp8 DR), k2-outer so k2=0 runs on half of g8 ---
            b8 = spool.tile([P, 4, 512], F8, tag="b8", name="b8_sb")
            b_ps = [pg.tile([P, 512], F32, tag="acc", name=f"b_ps{m}") for m in range(4)]
            for k2 in range(2):
                for m in range(4):
                    nc.tensor.matmul(
                        b_ps[m][:],
                        lhsT=g8[:, 2 * k2:2 * k2 + 2, m * P:(m + 1) * P],
                        rhs=wv_sb[:, 2 * k2:2 * k2 + 2, :],
                        start=(k2 == 0), stop=(k2 == 1), perf_mode=DR)
            for m in range(4):
                if m % 2 == 0:
                    nc.vector.tensor_copy(b8[:, m, :], b_ps[m][:])
                else:
                    nc.scalar.copy(b8[:, m, :], b_ps[m][:])

            # --- FP = B^T @ Wk diag windows (fp8 DR) -> d8 = D*2^-12 ---
            def fp_phase(m):
                fp_ps = pt.tile([P, P], F32, tag="tp", name="fp_ps")
                for k2 in range(2):
                    nc.tensor.matmul(
                        fp_ps[:],
                        lhsT=b8[:, 2 * k2:2 * k2 + 2, m * P:(m + 1) * P],
                        rhs=wk_sb[:, 2 * k2:2 * k2 + 2, m * P:(m + 1) * P],
                        start=(k2 == 0), stop=(k2 == 1), perf_mode=DR)
                h0, h1 = 2 * m, 2 * m + 1
                nc.vector.tensor_scalar_mul(
                    d8[0:64, m, 64 * h0:64 * h0 + 64], fp_ps[0:64, 0:64], 2.0 ** -6)
                nc.scalar.activation(
                    d8[64:128, m, 64 * h1:64 * h1 + 64], fp_ps[64:128, 64:128],
                    COPY, scale=2.0 ** -6)

            # --- N = blockdiag(D) @ Wo, all fp8 (no upcast): Ns*2^-12 ---
            ns8 = spool.tile([P, 4, 512], F8, tag="ns8", name="ns8_sb")

            def n_phase(m):
                n_ps = pt.tile([P, 512], F32, tag="tp", name="n_ps")
                nc.tensor.matmul(
                    n_ps[:],
                    lhsT=d8[:, m, m * P:(m + 1) * P],
                    rhs=wo_sb[:, m, :],
                    start=True, stop=True)
                if m % 2 == 0:
                    nc.vector.tensor_scalar_mul(ns8[:, m, :], n_ps[:], 2.0 ** -5)
                else:
                    nc.scalar.activation(ns8[:, m, :], n_ps[:], COPY, scale=2.0 ** -5)

            for m in range(4):
                fp_phase(m)
            for m in range(4):
                n_phase(m)

            # --- M = Wqt^T @ Ns (fp8 DR), digits m8hi/m8lo = M*2^-24 ---
            m8hi = spool.tile([P, 4, 512], F8, tag="m8hi", name="m8hi_sb")
            m8lo = spool.tile([P, 4, 512], F8, tag="m8lo", name="m8lo_sb")
            m_ps = [pg.tile([P, 512], F32, tag="acc", name=f"m_ps{m}") for m in range(4)]
            for k2 in range(2):
                for m in range(4):
                    nc.tensor.matmul(
                        m_ps[m][:],
                        lhsT=wqt_sb[:, 2 * k2:2 * k2 + 2, m * P:(m + 1) * P],
                        rhs=ns8[:, 2 * k2:2 * k2 + 2, :],
                        start=(k2 == 0), stop=(k2 == 1), perf_mode=DR)
            for m in range(4):
                if m % 2 == 1:
                    nc.scalar.activation(m8hi[:, m, :], m_ps[m][:], COPY, scale=2.0 ** -7)
                else:
                    nc.vector.tensor_scalar_mul(m8hi[:, m, :], m_ps[m][:], 2.0 ** -7)
            for m in range(4):
                # DVE-only; issued before the o2 copies so they drain while
                # the out phase runs its m8hi-only groups
                nc.vector.scalar_tensor_tensor(
                    m8lo[:, m, :], m_ps[m][:], 2.0 ** -7, m8hi[:, m, :],
                    mybir.AluOpType.mult, mybir.AluOpType.subtract)

            # --- out = xh@mh + xl@mh + xh@ml (fp8 DR, 6 accum groups/lb) ---
            o_ps = []
            for lb in range(8):
                pool, tag = (pg, "acc") if lb < 4 else (pt, "tp")
                o_ps.append(pool.tile([P, 512], F32, tag=tag, name=f"o_ps{lb}"))

            # gi 0,1 = (hi,hi); 2,3 = (lo,hi); 4,5 = (hi,lo)
            groups = [
                (xthi_sb, m8hi, 0), (xthi_sb, m8hi, 1),
                (xtlo_sb, m8hi, 0), (xtlo_sb, m8hi, 1),
                (xthi_sb, m8lo, 0), (xthi_sb, m8lo, 1),
            ]

            def omm(lb, gi):
                xsrc, msrc, k2 = groups[gi]
                nc.tensor.matmul(
                    o_ps[lb][:],
                    lhsT=xsrc[:, k2, :, lb * P:(lb + 1) * P],
                    rhs=msrc[:, 2 * k2:2 * k2 + 2, :],
                    start=(gi == 0), stop=(gi == 5), perf_mode=DR)


            orr = out_d.rearrange("(q p) d -> p q d", p=P)  # [128, 8, 512]

            def finish_lb(lb, o_t, slot):
                for gi in range(6):
                    omm(lb, gi)
                if lb >= 6:  # split the last copies across both engines
                    nc.vector.tensor_copy(o_t[:, slot, 0:256], o_ps[lb][:, 0:256])
                    nc.scalar.copy(o_t[:, slot, 256:512], o_ps[lb][:, 256:512])
                elif lb % 2 == 0:
                    nc.vector.tensor_copy(o_t[:, slot, :], o_ps[lb][:])
                else:
                    nc.scalar.copy(o_t[:, slot, :], o_ps[lb][:])

            for pair in range(4):  # lb-major tail, stores trickle per pair
                o2 = opool.tile([P, 2, 512], F16, tag=f"o{pair}", name=f"o2_{pair}")
                finish_lb(2 * pair, o2, 0)
                finish_lb(2 * pair + 1, o2, 1)
                nc.sync.dma_start(out=orr[:, 2 * pair:2 * pair + 2, :], in_=o2[:])

    nc.compile()
    return nc


def _get_nc():
    if "nc" not in _CACHE:
        _CACHE["nc"] = _build()
    return _CACHE["nc"]


def _pack_xt_digits(xc):
    xt = np.ascontiguousarray(xc.T)                  # (D, CHUNK) f32
    hi = xt.astype(NPF8)
    lo = (xt - hi.astype(np.float32)).astype(NPF8)

    def pack(a):
        return np.ascontiguousarray(
            a.reshape(2, 2, P, CHUNK).transpose(2, 0, 1, 3).reshape(P, 4 * CHUNK))
    return pack(hi), pack(lo)


def kernel(x, W_q, W_k, W_v, W_o):
    x = np.ascontiguousarray(np.asarray(x, np.float32))
    W_q = np.asarray(W_q, np.float32)
    W_k = np.asarray(W_k, np.float32)
    W_v = np.asarray(W_v, np.float32)
    W_o = np.asarray(W_o, np.float32)

    wv8 = np.ascontiguousarray(W_v.transpose(1, 0, 2).reshape(D, D)).astype(NPF8)
    wk8 = np.ascontiguousarray(W_k.transpose(1, 0, 2).reshape(D, D)).astype(NPF8)
    wqt8 = np.ascontiguousarray(W_q.transpose(0, 2, 1).reshape(D, D)).astype(NPF8)
    wo8 = np.ascontiguousarray(W_o.reshape(D, D)).astype(NPF8)

    nc = _get_nc()
    in_maps = []
    for i in range(N_CORES):
        b, c = divmod(i, 4)
        xb = np.roll(x[b], -c * CHUNK, axis=0)
        x8 = np.ascontiguousarray(xb).astype(NPF8)
        xthi, xtlo = _pack_xt_digits(x[b, c * CHUNK:(c + 1) * CHUNK])
        in_maps.append(
            {"x8": x8, "xthi": xthi, "xtlo": xtlo,
             "wv": wv8, "wk": wk8, "wo": wo8, "wqt": wqt8}
        )

    res = run_bass_kernel_spmd(nc, in_maps, list(range(N_CORES)))

    out = np.empty((B, L, D), np.float32)
    for i in range(N_CORES):
        b, c = divmod(i, 4)
        out[b, c * CHUNK:(c + 1) * CHUNK] = res.results[i]["out"].astype(np.float32) * 2.0 ** 24
    return out


# revision 5
# speedup vs baseline: 1.0406x; 1.0006x over previous
"""Bilinear (softmax-free) multi-head attention on 8 TRN2 NeuronCores.

Math: for each batch b,
    out_b = x_b @ M_b,   M_b = sum_h Wq[h] @ (Wk[h].T @ (x_b.T x_b) @ Wv[h]) @ Wo[h]
since (Q K^T) V = Q (K^T V) and every projection is linear: the O(L^2)
attention collapses to one L-sized Gram matrix G = x^T x, a tiny 512x512
head-folding chain, and one L-sized output GEMM.

Distribution (SPMD, no collectives): core i handles batch b = i//4 and output
row chunk c = i%4. Each core streams the full x_b to build G redundantly,
folds all 8 heads into M, and computes/stores only its own 1024-row out
slice; the per-core x is row-rotated so all 8 cores share one program.

fp8 (e4m3) strategy — DoubleRow perf mode (2 stacked k-tiles per matmul,
0.5 PE cycles/output column, 4x fp16 MAC rate):
  - G: 16 DoubleRow passes of 256 rows, upper-triangle blocks only; lower
    blocks mirrored by PE transpose (fp8 transpose writes element-step-2).
  - chain B = G@Wv, FP = B^T@Wk (diag windows -> D_h^T), N = blockdiag(D)@Wo,
    M = Wqt^T@Ns: all fp8 (B/FP/M DoubleRow).
  - out phase: two-digit fp8: x_chunk^T and M each split into hi+lo fp8
    digits (lo = requantized residual); out = xh@mh + xl@mh + xh@ml.
  - end-to-end rel err ~9.1e-3 vs the 2e-2 gate.

Schedule (TimelineSim-driven): x8 chunks big-first so the DMA stream is
gapless and G (supply-bound) ends right behind the last chunk; weights
follow in chain-stage order; ACT's activation table is preloaded at t~0;
FP phases all precede N phases so d8 copies aren't queued behind ns8; the
out phase runs fully lb-major so PSUM->SBUF drains + pair stores trickle
out behind the matmuls.

Scale ledger (all power-of-2, folded host-side or into scaled copies):
g8 = G*2^-6, b8 = B*2^-6, d8 = D*2^-12, ns8 = Ns*2^-17,
m8 digits = M*2^-24, out = out*2^-24 (host multiplies back 2^24).
"""

import numpy as np
import ml_dtypes

import concourse.tile as tile
from concourse import bacc, mybir
from concourse.bass_utils import run_bass_kernel_spmd
from concourse.masks import make_identity

F32 = mybir.dt.float32
F16 = mybir.dt.float16
F8 = mybir.dt.float8e4
DR = mybir.MatmulPerfMode.DoubleRow
COPY = mybir.ActivationFunctionType.Copy
NPF8 = ml_dtypes.float8_e4m3

B, L, D = 2, 4096, 512
H, DK = 8, 64
CHUNK = 1024
P = 128
TL = L // P            # 32 x-tiles of 128 rows
NT2 = TL // 2          # 16 DoubleRow passes
N_CORES = 8
N_WARM = 5
X_CHUNKS = [(0, 6), (6, 12), (12, 18), (18, 24), (24, 28), (28, 30), (30, 32)]

_CACHE = {}


def _build():
    nc = bacc.Bacc("TRN2", target_bir_lowering=False, debug=False)

    x8_d = nc.dram_tensor("x8", [L, D], F8, kind="ExternalInput").ap()
    xthi_d = nc.dram_tensor("xthi", [P, 4096], F8, kind="ExternalInput").ap()
    xtlo_d = nc.dram_tensor("xtlo", [P, 4096], F8, kind="ExternalInput").ap()
    wv_d = nc.dram_tensor("wv", [D, D], F8, kind="ExternalInput").ap()
    wk_d = nc.dram_tensor("wk", [D, D], F8, kind="ExternalInput").ap()
    wo_d = nc.dram_tensor("wo", [D, D], F8, kind="ExternalInput").ap()
    wqt_d = nc.dram_tensor("wqt", [D, D], F8, kind="ExternalInput").ap()
    out_d = nc.dram_tensor("out", [CHUNK, D], F16, kind="ExternalOutput").ap()

    with tile.TileContext(nc) as tc:
        import contextlib

        with contextlib.ExitStack() as ctx:
            consts = ctx.enter_context(tc.tile_pool(name="consts", bufs=1))
            wpool = ctx.enter_context(tc.tile_pool(name="wpool", bufs=1))
            xpool = ctx.enter_context(tc.tile_pool(name="xpool", bufs=1))
            spool = ctx.enter_context(tc.tile_pool(name="spool", bufs=1))
            opool = ctx.enter_context(tc.tile_pool(name="opool", bufs=5))
            pg = ctx.enter_context(tc.tile_pool(name="pg", bufs=4, space="PSUM"))
            pt = ctx.enter_context(tc.tile_pool(name="pt", bufs=4, space="PSUM"))

            # PE warmup: starts the pstate busy streak at t~0.
            warm = consts.tile([P, 512], F16, tag="warm")
            nc.vector.memset(warm[:], 0.0)
            # dummy ACT op at t~0 so the 1.3us LoadActFuncSet runs during the
            # DMA lead-in instead of blocking the first real scaled copy
            nc.scalar.activation(warm[:, 0:8], warm[:, 0:8], COPY, scale=1.0)
            wps = pg.tile([P, 512], F32, tag="acc", name="warm_ps")
            for _ in range(N_WARM):
                nc.tensor.matmul(wps[:], lhsT=warm[:, 0:P], rhs=warm[:],
                                 start=True, stop=True)

            ident_f32 = consts.tile([P, P], F32, tag="identf")
            make_identity(nc, ident_f32)
            ident8 = consts.tile([P, P], F8, tag="ident8")
            nc.vector.tensor_copy(ident8[:], ident_f32[:])

            # fp8 block-diag D^T holder, zero-filled off the critical path
            d8 = spool.tile([P, 4, 512], F8, tag="d8", name="d8_sb")
            nc.gpsimd.memset(d8[:], 0.0)

            # --- x8 stream into one [128, 32, 512] tile; G pass t2 reads
            # slice [:, 2t2:2t2+2, :], chunk boundaries are pass-aligned ---
            xr = x8_d.rearrange("(t p) d -> p t d", p=P)
            x8_sb = xpool.tile([P, TL, 512], F8, tag="x8", name="x8_sb")
            for a, b in X_CHUNKS:
                nc.sync.dma_start(out=x8_sb[:, a:b, :], in_=xr[:, a:b, :])

            # --- weights: 4 single fp8 transfers in chain-stage order ---
            wv_sb = wpool.tile([P, 4, 512], F8, tag="wv", name="wv_sb")
            wk_sb = wpool.tile([P, 4, 512], F8, tag="wk", name="wk_sb")
            wo_sb = wpool.tile([P, 4, 512], F8, tag="wo", name="wo_sb")
            wqt_sb = wpool.tile([P, 4, 512], F8, tag="wqt", name="wqt_sb")
            for sb, dram in ((wv_sb, wv_d), (wk_sb, wk_d), (wo_sb, wo_d), (wqt_sb, wqt_d)):
                nc.sync.dma_start(out=sb[:], in_=dram.rearrange("(c p) j -> p c j", p=P))

            # --- own-chunk x^T fp8 digits [p, kc2, j, l] ---
            xthi_sb = spool.tile([P, 2, 2, 1024], F8, tag="xthi", name="xthi_sb")
            xtlo_sb = spool.tile([P, 2, 2, 1024], F8, tag="xtlo", name="xtlo_sb")
            nc.sync.dma_start(
                out=xthi_sb[:], in_=xthi_d.rearrange("p (k j l) -> p k j l", k=2, j=2))
            nc.sync.dma_start(
                out=xtlo_sb[:], in_=xtlo_d.rearrange("p (k j l) -> p k j l", k=2, j=2))

            # --- G = x^T x, fp8 DoubleRow, upper-triangle blocks only ---
            g_n0 = [0, 128, 256, 384]
            g_ps = [pg.tile([P, 512], F32, tag="acc", name=f"g_ps{m}") for m in range(4)]
            for t2 in range(NT2 - 1):
                xp = x8_sb[:, 2 * t2:2 * t2 + 2, :]
                for m in range(4):
                    n0 = g_n0[m]
                    nc.tensor.matmul(
                        g_ps[m][:, n0:512],
                        lhsT=xp[:, :, m * P:(m + 1) * P], rhs=xp[:, :, n0:512],
                        start=(t2 == 0), stop=False, perf_mode=DR)
            xp = x8_sb[:, TL - 2:TL, :]
            for m in range(4):  # final pass m-outer: g_ps[m] closes in sequence
                n0 = g_n0[m]
                nc.tensor.matmul(
                    g_ps[m][:, n0:512],
                    lhsT=xp[:, :, m * P:(m + 1) * P], rhs=xp[:, :, n0:512],
                    start=False, stop=True, perf_mode=DR)

            g8 = spool.tile([P, 4, 512], F8, tag="g8", name="g8_sb")

            def g_copy(m, eng):
                n0 = g_n0[m]
                if eng == "dve":
                    nc.vector.tensor_scalar_mul(g8[:, m, n0:512], g_ps[m][:, n0:512], 2.0 ** -6)
                else:
                    nc.scalar.activation(g8[:, m, n0:512], g_ps[m][:, n0:512], COPY, scale=2.0 ** -6)

            def g_mirror(mr, jc, eng):
                mir_ps = pt.tile([P, P, 2], F8, tag="tp", name="mir_ps")
                nc.tensor.transpose(mir_ps[:, :, 0], g8[:, jc, mr * P:(mr + 1) * P], ident8[:])
                eng(g8[:, mr, jc * P:(jc + 1) * P], mir_ps[:, :, 0])

            # upper-left quadrant first so B's k2=0 pass unblocks early
            g_copy(0, "dve")
            g_copy(1, "act")
            g_mirror(1, 0, nc.vector.tensor_copy)
            g_copy(2, "act")
            g_copy(3, "dve")
            g_mirror(2, 0, nc.scalar.copy)
            g_mirror(3, 0, nc.vector.tensor_copy)
            g_mirror(2, 1, nc.scalar.copy)
            g_mirror(3, 1, nc.vector.tensor_copy)
            g_mirror(3, 2, nc.scalar.copy)

            # --- B = G @ Wv (fp8 DR), k2-outer so k2=0 runs on half of g8 ---
            b8 = spool.tile([P, 4, 512], F8, tag="b8", name="b8_sb")
            b_ps = [pg.tile([P, 512], F32, tag="acc", name=f"b_ps{m}") for m in range(4)]
            for k2 in range(2):
                for m in range(4):
                    nc.tensor.matmul(
                        b_ps[m][:],
                        lhsT=g8[:, 2 * k2:2 * k2 + 2, m * P:(m + 1) * P],
                        rhs=wv_sb[:, 2 * k2:2 * k2 + 2, :],
                        start=(k2 == 0), stop=(k2 == 1), perf_mode=DR)
            for m in range(4):
                if m % 2 == 0:
                    nc.vector.tensor_copy(b8[:, m, :], b_ps[m][:])
                else:
                    nc.scalar.copy(b8[:, m, :], b_ps[m][:])

            # --- FP = B^T @ Wk diag windows (fp8 DR) -> d8 = D*2^-12 ---
            def fp_phase(m):
                fp_ps = pt.tile([P, P], F32, tag="tp", name="fp_ps")
                for k2 in range(2):
                    nc.tensor.matmul(
                        fp_ps[:],
                        lhsT=b8[:, 2 * k2:2 * k2 + 2, m * P:(m + 1) * P],
                        rhs=wk_sb[:, 2 * k2:2 * k2 + 2, m * P:(m + 1) * P],
                        start=(k2 == 0), stop=(k2 == 1), perf_mode=DR)
                h0, h1 = 2 * m, 2 * m + 1
                nc.vector.tensor_scalar_mul(
                    d8[0:64, m, 64 * h0:64 * h0 + 64], fp_ps[0:64, 0:64], 2.0 ** -6)
                nc.scalar.activation(
                    d8[64:128, m, 64 * h1:64 * h1 + 64], fp_ps[64:128, 64:128],
                    COPY, scale=2.0 ** -6)

            # --- N = blockdiag(D) @ Wo, all fp8 (no upcast): Ns*2^-12 ---
            ns8 = spool.tile([P, 4, 512], F8, tag="ns8", name="ns8_sb")

            def n_phase(m):
                n_ps = pt.tile([P, 512], F32, tag="tp", name="n_ps")
                nc.tensor.matmul(
                    n_ps[:],
                    lhsT=d8[:, m, m * P:(m + 1) * P],
                    rhs=wo_sb[:, m, :],
                    start=True, stop=True)
                if m % 2 == 0:
                    nc.vector.tensor_scalar_mul(ns8[:, m, :], n_ps[:], 2.0 ** -5)
                else:
                    nc.scalar.activation(ns8[:, m, :], n_ps[:], COPY, scale=2.0 ** -5)

            for m in range(4):
                fp_phase(m)
            for m in range(4):
                n_phase(m)

            # --- M = Wqt^T @ Ns (fp8 DR), digits m8hi/m8lo = M*2^-24 ---
            m8hi = spool.tile([P, 4, 512], F8, tag="m8hi", name="m8hi_sb")
            m8lo = spool.tile([P, 4, 512], F8, tag="m8lo", name="m8lo_sb")
            m_ps = [pg.tile([P, 512], F32, tag="acc", name=f"m_ps{m}") for m in range(4)]
            for k2 in range(2):
                for m in range(4):
                    nc.tensor.matmul(
                        m_ps[m][:],
                        lhsT=wqt_sb[:, 2 * k2:2 * k2 + 2, m * P:(m + 1) * P],
                        rhs=ns8[:, 2 * k2:2 * k2 + 2, :],
                        start=(k2 == 0), stop=(k2 == 1), perf_mode=DR)
            for m in range(4):
                if m % 2 == 1:
                    nc.scalar.activation(m8hi[:, m, :], m_ps[m][:], COPY, scale=2.0 ** -7)
                else:
                    nc.vector.tensor_scalar_mul(m8hi[:, m, :], m_ps[m][:], 2.0 ** -7)
            for m in range(4):
                # DVE-only; issued before the o2 copies so they drain while
                # the out phase runs its m8hi-only groups
                nc.vector.scalar_tensor_tensor(
                    m8lo[:, m, :], m_ps[m][:], 2.0 ** -7, m8hi[:, m, :],
                    mybir.AluOpType.mult, mybir.AluOpType.subtract)

            # --- out = xh@mh + xl@mh + xh@ml (fp8 DR, 6 accum groups/lb) ---
            o_ps = []
            for lb in range(8):
                pool, tag = (pg, "acc") if lb < 4 else (pt, "tp")
                o_ps.append(pool.tile([P, 512], F32, tag=tag, name=f"o_ps{lb}"))

            # gi 0,1 = (hi,hi); 2,3 = (lo,hi); 4,5 = (hi,lo)
            groups = [
                (xthi_sb, m8hi, 0), (xthi_sb, m8hi, 1),
                (xtlo_sb, m8hi, 0), (xtlo_sb, m8hi, 1),
                (xthi_sb, m8lo, 0), (xthi_sb, m8lo, 1),
            ]

            def omm(lb, gi):
                xsrc, msrc, k2 = groups[gi]
                nc.tensor.matmul(
                    o_ps[lb][:],
                    lhsT=xsrc[:, k2, :, lb * P:(lb + 1) * P],
                    rhs=msrc[:, 2 * k2:2 * k2 + 2, :],
                    start=(gi == 0), stop=(gi == 5), perf_mode=DR)


            orr = out_d.rearrange("(q p) d -> p q d", p=P)  # [128, 8, 512]

            def finish_lb(lb, o_t, slot):
                for gi in range(6):
                    omm(lb, gi)
                if lb % 2 == 0:
                    nc.vector.tensor_copy(o_t[:, slot, :], o_ps[lb][:])
                else:
                    nc.scalar.copy(o_t[:, slot, :], o_ps[lb][:])

            for pair in range(4):  # lb-major tail, stores trickle per pair
                o2 = opool.tile([P, 2, 512], F16, tag=f"o{pair}", name=f"o2_{pair}")
                finish_lb(2 * pair, o2, 0)
                finish_lb(2 * pair + 1, o2, 1)
                nc.sync.dma_start(out=orr[:, 2 * pair:2 * pair + 2, :], in_=o2[:])

    nc.compile()
    return nc


def _get_nc():
    if "nc" not in _CACHE:
        _CACHE["nc"] = _build()
    return _CACHE["nc"]


def _pack_xt_digits(xc):
    xt = np.ascontiguousarray(xc.T)                  # (D, CHUNK) f32
    hi = xt.astype(NPF8)
    lo = (xt - hi.astype(np.float32)).astype(NPF8)

    def pack(a):
        return np.ascontiguousarray(
            a.reshape(2, 2, P, CHUNK).transpose(2, 0, 1, 3).reshape(P, 4 * CHUNK))
    return pack(hi), pack(lo)


def kernel(x, W_q, W_k, W_v, W_o):
    x = np.ascontiguousarray(np.asarray(x, np.float32))
    W_q = np.asarray(W_q, np.float32)
    W_k = np.asarray(W_k, np.float32)
    W_v = np.asarray(W_v, np.float32)
    W_o = np.asarray(W_o, np.float32)

    wv8 = np.ascontiguousarray(W_v.transpose(1, 0, 2).reshape(D, D)).astype(NPF8)
    wk8 = np.ascontiguousarray(W_k.transpose(1, 0, 2).reshape(D, D)).astype(NPF8)
    wqt8 = np.ascontiguousarray(W_q.transpose(0, 2, 1).reshape(D, D)).astype(NPF8)
    wo8 = np.ascontiguousarray(W_o.reshape(D, D)).astype(NPF8)

    nc = _get_nc()
    in_maps = []
    for i in range(N_CORES):
        b, c = divmod(i, 4)
        xb = np.roll(x[b], -c * CHUNK, axis=0)
        x8 = np.ascontiguousarray(xb).astype(NPF8)
        xthi, xtlo = _pack_xt_digits(x[b, c * CHUNK:(c + 1) * CHUNK])
        in_maps.append(
            {"x8": x8, "xthi": xthi, "xtlo": xtlo,
             "wv": wv8, "wk": wk8, "wo": wo8, "wqt": wqt8}
        )

    res = run_bass_kernel_spmd(nc, in_maps, list(range(N_CORES)))

    out = np.empty((B, L, D), np.float32)
    for i in range(N_CORES):
        b, c = divmod(i, 4)
        out[b, c * CHUNK:(c + 1) * CHUNK] = res.results[i]["out"].astype(np.float32) * 2.0 ** 24
    return out
